# revision 7
# baseline (speedup 1.0000x reference)
"""DistogramHead Trainium2 kernel v3 (host s-rows, device = add+sat-convert+store).

out[b, i, j] = relu(0.5*(s_i[b,i] + s_j[b,j]) + b_out); s_i/s_j are per-token
scalars. Host computes them in f32 (it already must, for the quantization
scale bound) and ships, per core:
  rbb (128, 4096) bf16 : bf16((s_j - mid)*inv), pre-broadcast across partitions
  ac  (128, 16)   f32  : a[p,u] = (s_i[u*128+p] + const)*inv + mid
Device computes q[u*128+p, j] = sat_u8(rne(rb[p,j] + ac[p,u])) -- the f32->u8
convert saturates negatives to 0 (measured bit-exact == clip(rint,0,255)), so
relu comes free with the convert and DVE ops are add-only. Host dequantizes
q*scale. Centering s_j at mid halves bf16 rounding error of rb.

Sharding: core c -> batch b=c//2, row half r=c%2 -> out[b, r*2048:(r+1)*2048, :].

Measured rates (this part): DVE add (128,4096)->u8 ~2290 ns, (128,2048) ~1220;
ACT activation (128,4096) ~3694, (128,2048) ~1988. Split: DVE 10 row-blocks /
ACT 6. First block per engine is 2x2048-wide so compute starts as soon as the
first rb half lands. All 16 out tiles persistent (no pool recycling); all
stores on the sync queue, enqueued in predicted completion order.
"""

import numpy as np

B = 4
L = 4096
D = 256
P = 128
NCORES = 8
ROWS_PER_CORE = L // 2          # 2048
NBLK = ROWS_PER_CORE // P       # 16
HALF = L // 2                   # 2048

_PROGRAM = None


def _build_program_raw():
    """Raw-bass build (no TileContext): manual semaphores, no tile
    entry/exit barriers (~4-5us cheaper pre/postamble)."""
    import concourse.bacc as bacc
    from concourse import mybir

    f32 = mybir.dt.float32
    bf16 = mybir.dt.bfloat16
    u8 = mybir.dt.uint8
    nc = bacc.Bacc(None)

    rbb = nc.dram_tensor("rbb", [P, L], bf16, kind="ExternalInput")
    ac = nc.dram_tensor("ac", [P, NBLK], f32, kind="ExternalInput")
    out = nc.dram_tensor("out", [NBLK, P, L], u8, kind="ExternalOutput")

    Relu = mybir.ActivationFunctionType.Relu
    add = mybir.AluOpType.add

    rb = nc.alloc_sbuf_tensor("rb", [P, L], bf16)
    acs = nc.alloc_sbuf_tensor("acs", [P, NBLK], f32)
    scr = nc.alloc_sbuf_tensor("scr", [P, 2], bf16)
    scro = nc.alloc_sbuf_tensor("scro", [P, 2], u8)
    ots = [nc.alloc_sbuf_tensor(f"ot{u}", [P, L], u8) for u in range(NBLK)]

    s_lg = nc.alloc_semaphore("s_lg")   # gpsimd queue loads: acs, rb[2048:]
    s_ls = nc.alloc_semaphore("s_ls")   # sync queue load: rb[0:1024]
    s_lc = nc.alloc_semaphore("s_lc")   # scalar queue load: rb[1024:2048]
    s_dve = nc.alloc_semaphore("s_dve")
    s_act = nc.alloc_semaphore("s_act")
    s_sts = nc.alloc_semaphore("s_sts")  # sync-queue store completions
    s_stg = nc.alloc_semaphore("s_stg")  # gpsimd-queue store completions
    s_w = nc.alloc_semaphore("s_w")      # scratch memset -> warmup act

    # ACT relu-table warmup: gpsimd memset feeds the activation
    nc.gpsimd.memset(scr.ap(), 0.0).then_inc(s_w, 1)
    nc.scalar.wait_ge(s_w, 1)
    nc.scalar.activation(scro.ap(), scr.ap(), Relu, scale=1.0)

    # loads in parallel on three queues
    nc.gpsimd.dma_start(out=acs[:, :], in_=ac[:, :]).then_inc(s_lg, 16)
    nc.sync.dma_start(out=rb[:, 0:1024], in_=rbb[:, 0:1024]).then_inc(s_ls, 16)
    nc.scalar.dma_start(out=rb[:, 1024:HALF], in_=rbb[:, 1024:HALF]).then_inc(s_lc, 16)
    nc.gpsimd.dma_start(out=rb[:, HALF:L], in_=rbb[:, HALF:L]).then_inc(s_lg, 16)

    ndve = [0]
    nact = [0]

    def dve_op(u, j0, w):
        nc.vector.tensor_scalar(
            out=ots[u][:, j0:j0 + w], in0=rb[:, j0:j0 + w],
            scalar1=acs[:, u:u + 1], scalar2=None, op0=add,
        ).then_inc(s_dve, 1)
        ndve[0] += 1
        return ndve[0]

    def act_op(u, j0, w):
        nc.scalar.activation(
            ots[u][:, j0:j0 + w], rb[:, j0:j0 + w], Relu,
            bias=acs[:, u:u + 1], scale=1.0,
        ).then_inc(s_act, 1)
        nact[0] += 1
        return nact[0]

    nst = [0, 0]

    def store(sem, k, u, j0=0, w=L):
        # DVE tiles stored via sync queue, ACT tiles via gpsimd queue
        if sem is s_dve:
            nc.sync.wait_ge(sem, k)
            nc.sync.dma_start(out=out[u, :, j0:j0 + w],
                              in_=ots[u][:, j0:j0 + w]).then_inc(s_sts, 16)
            nst[0] += 16
        else:
            nc.gpsimd.wait_ge(sem, k)
            nc.gpsimd.dma_start(out=out[u, :, j0:j0 + w],
                                in_=ots[u][:, j0:j0 + w]).then_inc(s_stg, 16)
            nst[1] += 16

    # DVE stream: u0 in chunks following the loads, then wides u1..u9
    nc.vector.wait_ge(s_lg, 16)     # acs
    nc.vector.wait_ge(s_ls, 16)     # rb[0:1024]
    k1 = dve_op(0, 0, 1024)
    nc.vector.wait_ge(s_lc, 16)     # rb[1024:2048]
    k2 = dve_op(0, 1024, 1024)
    nc.vector.wait_ge(s_lg, 32)     # rb[2048:4096]
    dve_ks = [dve_op(0, HALF, HALF)]
    for u in range(1, 9):
        dve_ks.append(dve_op(u, 0, L))
    k_last1 = dve_op(9, 0, HALF)
    k_last2 = dve_op(9, HALF, 1024)
    k_last3 = dve_op(9, 3072, 1024)

    # ACT stream: u15 in chunks, then wides u14..u10
    nc.scalar.wait_ge(s_lg, 16)
    nc.scalar.wait_ge(s_ls, 16)
    a1 = act_op(15, 0, 1024)
    nc.scalar.wait_ge(s_lc, 16)
    a2 = act_op(15, 1024, 1024)
    nc.scalar.wait_ge(s_lg, 32)
    act_ks = [act_op(15, HALF, HALF)]
    for u in range(14, 10, -1):
        act_ks.append(act_op(u, 0, L))
    a_last1 = act_op(10, 0, HALF)
    a_last2 = act_op(10, HALF, HALF)

    # stores: sync handles DVE tiles in production order
    nc.sync.wait_ge(s_dve, k2)
    nc.sync.dma_start(out=out[0, :, 0:HALF], in_=ots[0][:, 0:HALF]).then_inc(s_sts, 16)
    nst[0] += 16
    store(s_dve, dve_ks[0], 0, HALF, HALF)
    for i, u in enumerate(range(1, 9)):
        store(s_dve, dve_ks[1 + i], u)
    store(s_dve, k_last1, 9, 0, HALF)
    store(s_dve, k_last2, 9, HALF, 1024)
    store(s_dve, k_last3, 9, 3072, 1024)

    # gpsimd handles ACT tiles in production order
    nc.gpsimd.wait_ge(s_act, a2)
    nc.gpsimd.dma_start(out=out[15, :, 0:HALF], in_=ots[15][:, 0:HALF]).then_inc(s_stg, 16)
    nst[1] += 16
    store(s_act, act_ks[0], 15, HALF, HALF)
    for i, u in enumerate(range(14, 10, -1)):
        store(s_act, act_ks[1 + i], u)
    store(s_act, a_last1, 10, 0, HALF)
    store(s_act, a_last2, 10, HALF, HALF)

    # drain: each trigger engine waits for its own queue's completions
    nc.sync.wait_ge(s_sts, nst[0])
    nc.gpsimd.wait_ge(s_stg, nst[1])

    nc.finalize()
    return nc


def _build_program():
    import concourse.bacc as bacc
    import concourse.tile as tile
    from concourse import mybir

    f32 = mybir.dt.float32
    bf16 = mybir.dt.bfloat16
    u8 = mybir.dt.uint8
    nc = bacc.Bacc(None)

    rbb = nc.dram_tensor("rbb", [P, L], bf16, kind="ExternalInput")
    ac = nc.dram_tensor("ac", [P, NBLK], f32, kind="ExternalInput")
    out = nc.dram_tensor("out", [NBLK, P, L], u8, kind="ExternalOutput")

    Relu = None  # set below
    with tile.TileContext(nc) as tc:
        with tc.tile_pool(name="persist", bufs=1) as persist:
            Relu = mybir.ActivationFunctionType.Relu
            add = mybir.AluOpType.add

            rb = persist.tile([P, L], bf16, tag="rb")
            a_cols = persist.tile([P, NBLK], f32, tag="ac")
            scratch = persist.tile([P, 2], bf16, tag="scr")
            scr_out = persist.tile([P, 2], u8, tag="scro")
            ots = [persist.tile([P, L], u8, tag=f"ot{u}", name=f"ot{u}")
                   for u in range(NBLK)]

            # ACT relu-table preload during the rb DMA
            nc.vector.memset(scratch[:], 0.0)
            nc.scalar.activation(scr_out[:], scratch[:], Relu, scale=1.0)

            # a_cols on gpsimd queue (parallel trigger); rb in growing chunks
            # on sync so the first compute sliver can start ~9.6us.
            nc.gpsimd.dma_start(out=a_cols[:], in_=ac[:, :])
            nc.sync.dma_start(out=rb[:, 0:512], in_=rbb[:, 0:512])
            nc.sync.dma_start(out=rb[:, 512:1024], in_=rbb[:, 512:1024])
            nc.sync.dma_start(out=rb[:, 1024:HALF], in_=rbb[:, 1024:HALF])
            nc.sync.dma_start(out=rb[:, HALF:L], in_=rbb[:, HALF:L])

            # DVE: u0 (2x2048), then u1..u9 wide.  ACT: u15 (2x2048), then
            # u14..u10 wide.  Emission interleaved so each engine's stream
            # is in order; stores enqueued on sync in predicted completion
            # order (DVE tile every ~2.35us, ACT every ~3.69us).
            def dve_op(u, j0, w):
                nc.vector.tensor_scalar(
                    out=ots[u][:, j0:j0 + w], in0=rb[:, j0:j0 + w],
                    scalar1=a_cols[:, u:u + 1], scalar2=None, op0=add)

            def act_op(u, j0, w):
                nc.scalar.activation(
                    ots[u][:, j0:j0 + w], rb[:, j0:j0 + w], Relu,
                    bias=a_cols[:, u:u + 1], scale=1.0)

            def store(u, j0=0, w=L):
                nc.sync.dma_start(out=out[u, :, j0:j0 + w],
                                  in_=ots[u][:, j0:j0 + w])

            # sliver ops on block u0 (DVE) / u15 (ACT) matching the chunked
            # rb loads, so compute starts on the first 512 cols.
            for (j0, w) in [(0, 512), (512, 512), (1024, 1024), (HALF, HALF)]:
                dve_op(0, j0, w)
                act_op(15, j0, w)
            store(0, 0, HALF)
            store(15, 0, HALF)
            store(0, HALF, HALF)
            store(15, HALF, HALF)
            # wide middle blocks; interleave emission DVE:ACT ~ 3:2; final
            # block on each engine is split so the last store drains fast.
            emit = [('d', 1, [(0, L)]), ('a', 14, [(0, L)]),
                    ('d', 2, [(0, L)]), ('d', 3, [(0, L)]), ('a', 13, [(0, L)]),
                    ('d', 4, [(0, L)]), ('a', 12, [(0, L)]),
                    ('d', 5, [(0, L)]), ('d', 6, [(0, L)]), ('a', 11, [(0, L)]),
                    ('d', 7, [(0, L)]), ('a', 10, [(0, HALF), (HALF, HALF)]),
                    ('d', 8, [(0, L)]),
                    ('d', 9, [(0, HALF), (HALF, 1024), (3072, 1024)])]
            for kind, u, parts in emit:
                for (j0, w) in parts:
                    if kind == 'd':
                        dve_op(u, j0, w)
                    else:
                        act_op(u, j0, w)
                    store(u, j0, w)

    nc.finalize()
    return nc


def _get_program():
    global _PROGRAM
    if _PROGRAM is None:
        import os
        if os.environ.get("KRAW", "") == "1":
            _PROGRAM = _build_program_raw()
        else:
            _PROGRAM = _build_program()
    return _PROGRAM


def _run(inputs, trace=False):
    import ml_dtypes
    from concourse.bass_utils import run_bass_kernel_spmd

    bf16 = ml_dtypes.bfloat16
    x = np.asarray(inputs["x"], np.float32)
    w_i = np.asarray(inputs["w_i"], np.float32)
    w_j = np.asarray(inputs["w_j"], np.float32)
    b_i = np.asarray(inputs["b_i"], np.float32).reshape(-1)
    b_j = np.asarray(inputs["b_j"], np.float32).reshape(-1)
    w_out = np.asarray(inputs["w_out"], np.float32).reshape(-1)
    b_out = np.asarray(inputs["b_out"], np.float32).reshape(())

    # fold: out = relu(si2[i] + sj2[j] + const)
    v_i = 0.5 * (w_i @ w_out)
    v_j = 0.5 * (w_j @ w_out)
    const = np.float32(0.5 * (b_i @ w_out + b_j @ w_out) + b_out)
    si2 = x @ v_i                   # (B, L) f32
    sj2 = x @ v_j                   # (B, L) f32

    in_maps = []
    scales = []
    for c in range(NCORES):
        b, r = divmod(c, 2)
        si_slab = si2[b, r * ROWS_PER_CORE : (r + 1) * ROWS_PER_CORE] + const
        sj_row = sj2[b]
        gmax = float(si_slab.max() + sj_row.max())
        scale = np.float32(max(gmax, 1e-6) / 254.0)
        inv = np.float32(1.0 / scale)
        mid = np.float32(0.5 * (sj_row.max() + sj_row.min()) * inv)
        rb_row = (sj_row * inv - mid).astype(bf16)
        rbb = np.ascontiguousarray(np.broadcast_to(rb_row, (P, L)))
        acv = (si_slab * inv + mid).astype(np.float32)
        acm = np.ascontiguousarray(acv.reshape(NBLK, P).T)
        in_maps.append({"rbb": rbb, "ac": acm})
        scales.append(scale)

    nc = _get_program()
    res = run_bass_kernel_spmd(nc, in_maps, core_ids=list(range(NCORES)), trace=trace)
    full = np.empty((B, L, L), np.float32)
    for c in range(NCORES):
        b, r = divmod(c, 2)
        q = res.results[c]["out"].reshape(ROWS_PER_CORE, L)
        rows = slice(r * ROWS_PER_CORE, (r + 1) * ROWS_PER_CORE)
        full[b, rows, :] = q.astype(np.float32) * scales[c]
    return full, res


def kernel(**inputs):
    full, _ = _run(inputs, trace=False)
    return full


# revision 9
# speedup vs baseline: 1.0341x; 1.0341x over previous
"""DistogramHead Trainium2 kernel v3 (host s-rows, device = add+sat-convert+store).

out[b, i, j] = relu(0.5*(s_i[b,i] + s_j[b,j]) + b_out); s_i/s_j are per-token
scalars. Host computes them in f32 (it already must, for the quantization
scale bound) and ships, per core:
  rbb (128, 4096) bf16 : bf16((s_j - mid)*inv), pre-broadcast across partitions
  ac  (128, 16)   f32  : a[p,u] = (s_i[u*128+p] + const)*inv + mid
Device computes q[u*128+p, j] = sat_u8(rne(rb[p,j] + ac[p,u])) -- the f32->u8
convert saturates negatives to 0 (measured bit-exact == clip(rint,0,255)), so
relu comes free with the convert and DVE ops are add-only. Host dequantizes
q*scale. Centering s_j at mid halves bf16 rounding error of rb.

Sharding: core c -> batch b=c//2, row half r=c%2 -> out[b, r*2048:(r+1)*2048, :].

Measured rates (this part): DVE add (128,4096)->u8 ~2290 ns, (128,2048) ~1220;
ACT activation (128,4096) ~3694, (128,2048) ~1988. Split: DVE 10 row-blocks /
ACT 6. First block per engine is 2x2048-wide so compute starts as soon as the
first rb half lands. All 16 out tiles persistent (no pool recycling); all
stores on the sync queue, enqueued in predicted completion order.
"""

import numpy as np

B = 4
L = 4096
D = 256
P = 128
NCORES = 8
ROWS_PER_CORE = L // 2          # 2048
NBLK = ROWS_PER_CORE // P       # 16
HALF = L // 2                   # 2048

_PROGRAM = None


def _build_program_raw():
    """Raw-bass build (no TileContext): manual semaphores, no tile
    entry/exit barriers (~4-5us cheaper pre/postamble)."""
    import concourse.bacc as bacc
    from concourse import mybir

    f32 = mybir.dt.float32
    bf16 = mybir.dt.bfloat16
    u8 = mybir.dt.uint8
    nc = bacc.Bacc(None)

    rbb = nc.dram_tensor("rbb", [P, L], bf16, kind="ExternalInput")
    ac = nc.dram_tensor("ac", [P, NBLK], f32, kind="ExternalInput")
    out = nc.dram_tensor("out", [NBLK, P, L], u8, kind="ExternalOutput")

    Relu = mybir.ActivationFunctionType.Relu
    add = mybir.AluOpType.add

    rb = nc.alloc_sbuf_tensor("rb", [P, L], bf16)
    acs = nc.alloc_sbuf_tensor("acs", [P, NBLK], f32)
    scr = nc.alloc_sbuf_tensor("scr", [P, 2], bf16)
    scro = nc.alloc_sbuf_tensor("scro", [P, 2], u8)
    ots = [nc.alloc_sbuf_tensor(f"ot{u}", [P, L], u8) for u in range(NBLK)]

    s_lg = nc.alloc_semaphore("s_lg")   # gpsimd queue loads: acs, rb[2048:]
    s_ls = nc.alloc_semaphore("s_ls")   # sync queue load: rb[0:1024]
    s_lc = nc.alloc_semaphore("s_lc")   # scalar queue load: rb[1024:2048]
    s_dve = nc.alloc_semaphore("s_dve")
    s_act = nc.alloc_semaphore("s_act")
    s_sts = nc.alloc_semaphore("s_sts")  # sync-queue store completions
    s_stg = nc.alloc_semaphore("s_stg")  # gpsimd-queue store completions
    s_w = nc.alloc_semaphore("s_w")      # scratch memset -> warmup act

    # ACT relu-table warmup: gpsimd memset feeds the activation
    nc.gpsimd.memset(scr.ap(), 0.0).then_inc(s_w, 1)
    nc.scalar.wait_ge(s_w, 1)
    nc.scalar.activation(scro.ap(), scr.ap(), Relu, scale=1.0)

    # loads in parallel on three queues
    nc.gpsimd.dma_start(out=acs[:, :], in_=ac[:, :]).then_inc(s_lg, 16)
    nc.sync.dma_start(out=rb[:, 0:1024], in_=rbb[:, 0:1024]).then_inc(s_ls, 16)
    nc.scalar.dma_start(out=rb[:, 1024:HALF], in_=rbb[:, 1024:HALF]).then_inc(s_lc, 16)
    nc.gpsimd.dma_start(out=rb[:, HALF:L], in_=rbb[:, HALF:L]).then_inc(s_lg, 16)

    ndve = [0]
    nact = [0]

    def dve_op(u, j0, w):
        nc.vector.tensor_scalar(
            out=ots[u][:, j0:j0 + w], in0=rb[:, j0:j0 + w],
            scalar1=acs[:, u:u + 1], scalar2=None, op0=add,
        ).then_inc(s_dve, 1)
        ndve[0] += 1
        return ndve[0]

    def act_op(u, j0, w):
        nc.scalar.activation(
            ots[u][:, j0:j0 + w], rb[:, j0:j0 + w], Relu,
            bias=acs[:, u:u + 1], scale=1.0,
        ).then_inc(s_act, 1)
        nact[0] += 1
        return nact[0]

    nst = [0, 0]

    def store(sem, k, u, j0=0, w=L):
        # DVE tiles stored via sync queue, ACT tiles via gpsimd queue
        if sem is s_dve:
            nc.sync.wait_ge(sem, k)
            nc.sync.dma_start(out=out[u, :, j0:j0 + w],
                              in_=ots[u][:, j0:j0 + w]).then_inc(s_sts, 16)
            nst[0] += 16
        else:
            nc.gpsimd.wait_ge(sem, k)
            nc.gpsimd.dma_start(out=out[u, :, j0:j0 + w],
                                in_=ots[u][:, j0:j0 + w]).then_inc(s_stg, 16)
            nst[1] += 16

    # DVE stream: u0 in chunks following the loads, then wides u1..u9
    nc.vector.wait_ge(s_lg, 16)     # acs
    nc.vector.wait_ge(s_ls, 16)     # rb[0:1024]
    k1 = dve_op(0, 0, 1024)
    nc.vector.wait_ge(s_lc, 16)     # rb[1024:2048]
    k2 = dve_op(0, 1024, 1024)
    nc.vector.wait_ge(s_lg, 32)     # rb[2048:4096]
    dve_ks = [dve_op(0, HALF, HALF)]
    for u in range(1, 9):
        dve_ks.append(dve_op(u, 0, L))
    k_last1 = dve_op(9, 0, HALF)
    k_last2 = dve_op(9, HALF, 1024)
    k_last3 = dve_op(9, 3072, 1024)

    # ACT stream: u15 in chunks, then wides u14..u10
    nc.scalar.wait_ge(s_lg, 16)
    nc.scalar.wait_ge(s_ls, 16)
    a1 = act_op(15, 0, 1024)
    nc.scalar.wait_ge(s_lc, 16)
    a2 = act_op(15, 1024, 1024)
    nc.scalar.wait_ge(s_lg, 32)
    act_ks = [act_op(15, HALF, HALF)]
    for u in range(14, 10, -1):
        act_ks.append(act_op(u, 0, L))
    a_last1 = act_op(10, 0, HALF)
    a_last2 = act_op(10, HALF, HALF)

    # stores: sync handles DVE tiles in production order
    nc.sync.wait_ge(s_dve, k2)
    nc.sync.dma_start(out=out[0, :, 0:HALF], in_=ots[0][:, 0:HALF]).then_inc(s_sts, 16)
    nst[0] += 16
    store(s_dve, dve_ks[0], 0, HALF, HALF)
    for i, u in enumerate(range(1, 9)):
        store(s_dve, dve_ks[1 + i], u)
    store(s_dve, k_last1, 9, 0, HALF)
    store(s_dve, k_last2, 9, HALF, 1024)
    store(s_dve, k_last3, 9, 3072, 1024)

    # gpsimd handles ACT tiles in production order
    nc.gpsimd.wait_ge(s_act, a2)
    nc.gpsimd.dma_start(out=out[15, :, 0:HALF], in_=ots[15][:, 0:HALF]).then_inc(s_stg, 16)
    nst[1] += 16
    store(s_act, act_ks[0], 15, HALF, HALF)
    for i, u in enumerate(range(14, 10, -1)):
        store(s_act, act_ks[1 + i], u)
    store(s_act, a_last1, 10, 0, HALF)
    store(s_act, a_last2, 10, HALF, HALF)

    # drain: each trigger engine waits for its own queue's completions
    nc.sync.wait_ge(s_sts, nst[0])
    nc.gpsimd.wait_ge(s_stg, nst[1])

    nc.finalize()
    return nc


def _build_program():
    import concourse.bacc as bacc
    import concourse.tile as tile
    from concourse import mybir

    f32 = mybir.dt.float32
    bf16 = mybir.dt.bfloat16
    u8 = mybir.dt.uint8
    nc = bacc.Bacc(None)

    rbb = nc.dram_tensor("rbb", [P, L], bf16, kind="ExternalInput")
    ac = nc.dram_tensor("ac", [P, NBLK], f32, kind="ExternalInput")
    out = nc.dram_tensor("out", [NBLK, P, L], u8, kind="ExternalOutput")

    Relu = None  # set below
    with tile.TileContext(nc) as tc:
        with tc.tile_pool(name="persist", bufs=1) as persist:
            Relu = mybir.ActivationFunctionType.Relu
            add = mybir.AluOpType.add

            rb = persist.tile([P, L], bf16, tag="rb")
            a_cols = persist.tile([P, NBLK], f32, tag="ac")
            scratch = persist.tile([P, 2], bf16, tag="scr")
            scr_out = persist.tile([P, 2], u8, tag="scro")
            ots = [persist.tile([P, L], u8, tag=f"ot{u}", name=f"ot{u}")
                   for u in range(NBLK)]

            # ACT relu-table preload during the rb DMA
            nc.vector.memset(scratch[:], 0.0)
            nc.scalar.activation(scr_out[:], scratch[:], Relu, scale=1.0)

            # a_cols on gpsimd queue (parallel trigger); rb in growing chunks
            # on sync so the first compute sliver can start ~10.1us.
            nc.gpsimd.dma_start(out=a_cols[:], in_=ac[:, :])
            nc.sync.dma_start(out=rb[:, 0:1024], in_=rbb[:, 0:1024])
            nc.sync.dma_start(out=rb[:, 1024:HALF], in_=rbb[:, 1024:HALF])
            nc.sync.dma_start(out=rb[:, HALF:L], in_=rbb[:, HALF:L])

            # DVE: u0 (2x2048), then u1..u9 wide.  ACT: u15 (2x2048), then
            # u14..u10 wide.  Emission interleaved so each engine's stream
            # is in order; stores enqueued on sync in predicted completion
            # order (DVE tile every ~2.35us, ACT every ~3.69us).
            def dve_op(u, j0, w):
                nc.vector.tensor_scalar(
                    out=ots[u][:, j0:j0 + w], in0=rb[:, j0:j0 + w],
                    scalar1=a_cols[:, u:u + 1], scalar2=None, op0=add)

            def act_op(u, j0, w):
                nc.scalar.activation(
                    ots[u][:, j0:j0 + w], rb[:, j0:j0 + w], Relu,
                    bias=a_cols[:, u:u + 1], scale=1.0)

            def store(u, j0=0, w=L):
                nc.sync.dma_start(out=out[u, :, j0:j0 + w],
                                  in_=ots[u][:, j0:j0 + w])

            # Work split: DVE 39 units of 1024 cols (u0..u9 minus u9's last
            # sliver), ACT 25 (u15..u10 + u9's last sliver). Each (kind, u,
            # j0, w) below is one op followed immediately by its store on the
            # sync queue, emitted in predicted completion order so the FIFO
            # never head-of-line blocks.
            emit = [('d', 0, 0, 1024), ('a', 15, 0, 1024),
                    ('d', 0, 1024, 1024), ('a', 15, 1024, 1024),
                    ('d', 0, HALF, HALF), ('a', 15, HALF, HALF),
                    ('d', 1, 0, L), ('d', 2, 0, L), ('a', 14, 0, L),
                    ('d', 3, 0, L), ('a', 13, 0, L),
                    ('d', 4, 0, L), ('d', 5, 0, L), ('a', 12, 0, L),
                    ('d', 6, 0, L), ('a', 11, 0, L), ('d', 7, 0, L),
                    ('a', 10, 0, HALF), ('d', 8, 0, L),
                    ('d', 9, 0, HALF), ('a', 10, HALF, HALF),
                    ('d', 9, HALF, 1024), ('a', 9, 3072, 1024)]
            for kind, u, j0, w in emit:
                if kind == 'd':
                    dve_op(u, j0, w)
                else:
                    act_op(u, j0, w)
                store(u, j0, w)

    nc.finalize()
    return nc


def _get_program():
    global _PROGRAM
    if _PROGRAM is None:
        import os
        if os.environ.get("KRAW", "") == "1":
            _PROGRAM = _build_program_raw()
        else:
            _PROGRAM = _build_program()
    return _PROGRAM


def _run(inputs, trace=False):
    import ml_dtypes
    from concourse.bass_utils import run_bass_kernel_spmd

    bf16 = ml_dtypes.bfloat16
    x = np.asarray(inputs["x"], np.float32)
    w_i = np.asarray(inputs["w_i"], np.float32)
    w_j = np.asarray(inputs["w_j"], np.float32)
    b_i = np.asarray(inputs["b_i"], np.float32).reshape(-1)
    b_j = np.asarray(inputs["b_j"], np.float32).reshape(-1)
    w_out = np.asarray(inputs["w_out"], np.float32).reshape(-1)
    b_out = np.asarray(inputs["b_out"], np.float32).reshape(())

    # fold: out = relu(si2[i] + sj2[j] + const)
    v_i = 0.5 * (w_i @ w_out)
    v_j = 0.5 * (w_j @ w_out)
    const = np.float32(0.5 * (b_i @ w_out + b_j @ w_out) + b_out)
    si2 = x @ v_i                   # (B, L) f32
    sj2 = x @ v_j                   # (B, L) f32

    in_maps = []
    scales = []
    for c in range(NCORES):
        b, r = divmod(c, 2)
        si_slab = si2[b, r * ROWS_PER_CORE : (r + 1) * ROWS_PER_CORE] + const
        sj_row = sj2[b]
        gmax = float(si_slab.max() + sj_row.max())
        scale = np.float32(max(gmax, 1e-6) / 254.0)
        inv = np.float32(1.0 / scale)
        mid = np.float32(0.5 * (sj_row.max() + sj_row.min()) * inv)
        rb_row = (sj_row * inv - mid).astype(bf16)
        rbb = np.ascontiguousarray(np.broadcast_to(rb_row, (P, L)))
        acv = (si_slab * inv + mid).astype(np.float32)
        acm = np.ascontiguousarray(acv.reshape(NBLK, P).T)
        in_maps.append({"rbb": rbb, "ac": acm})
        scales.append(scale)

    nc = _get_program()
    res = run_bass_kernel_spmd(nc, in_maps, core_ids=list(range(NCORES)), trace=trace)
    full = np.empty((B, L, L), np.float32)
    for c in range(NCORES):
        b, r = divmod(c, 2)
        q = res.results[c]["out"].reshape(ROWS_PER_CORE, L)
        rows = slice(r * ROWS_PER_CORE, (r + 1) * ROWS_PER_CORE)
        full[b, rows, :] = q.astype(np.float32) * scales[c]
    return full, res


def kernel(**inputs):
    full, _ = _run(inputs, trace=False)
    return full
